# revision 24
# baseline (speedup 1.0000x reference)
"""CRF log-likelihood kernel for Trainium2 (Bass/Tile), 8-core data parallel.

out[b] = gold_path_score(b) - logZ(b)

logZ via exp-domain DP, fwd and bwd chains meeting at t = 256:
  fwd:  u_k    = el_k ⊙ (Wf^T u_{k-1}),   k = 1..256   (u_0 = el_0)
  bwd:  γ̃_σ   = el_{T-σ} ⊙ (Wb^T γ̃_{σ-1}), σ = 1..255  (γ̃_0 = sink one)
  Z    = Σ_j u_256[j] · (Wb^T γ̃_255)[j]   (len > 256; sink capture else)
Both chains have the same mm-then-mul dataflow, so all work is 4 uniform
"lanes" of 32 columns against ONE block-diagonal stationary
S = diag(WfG, WfG, WbG) loaded into the PE array exactly once
(ldweights=False on every chain matmul). Each tick: 4 matmuls into one
shared PSUM tile + 4 DVE multiplies into one shared state tile, so the
framework emits ~1 WAR guard per tick instead of 4.

Partition layout (99 rows): [0:33]=fwd block A (32 labels + sink),
[33:66]=fwd block B, [66:99]=bwd block C. Columns: lane l = cols
32l..32l+32. Fwd state of seq s lives at (lane s//64, block (s%64)//32,
col s%32); bwd state of seq s at (lane s//32, block C, col s%32).
Emissions are host-packed per (row, k, col): rows 0-65 carry fwd times k,
rows 66-98 carry bwd times T-k — one ascending-k DMA/exp stream feeds
both chains. No renorm: CSHIFT centers the per-tick log-drift at ~0 and
the random walk stays inside bf16/f32 exponent range. Host adds
CSHIFT*len and does the gold-path gathers / final subtract.
"""

import numpy as np
import ml_dtypes

B, T, L = 1024, 512, 32
NCORES = 8
BPC = B // NCORES        # 128 sequences per core
LN = 99                  # partitions: 3 blocks x (32 labels + 1 sink)
NCOLS = 128              # columns = sequences per core
NLANE = 4
CWID = 32                # columns per lane
KT = 256                 # chain ticks
TEXK = 257               # el slices k = 0..256
MEET = 256               # host: len <= MEET -> sink path, else combine
CSHIFT = 4.5
EL_WINDOWS = [(0, 4), (4, 8), (12, 16), (28, 32), (60, 64), (124, 64), (188, 64), (252, 5)]
STAGE_MAX = max(n for _, n in EL_WINDOWS)

_prog_cache = {}
last_result = None       # BassKernelResults of the most recent run (for test.py)


def _build_program():
    import concourse.bacc as bacc
    import concourse.tile as tile
    from concourse import mybir

    f32 = mybir.dt.float32
    bf16 = mybir.dt.bfloat16
    AF = mybir.ActivationFunctionType

    nc = bacc.Bacc("TRN2", target_bir_lowering=False, debug=False, num_devices=NCORES)
    lg = nc.dram_tensor("lg", [LN, TEXK, NCOLS], bf16, kind="ExternalInput")
    sm = nc.dram_tensor("sm", [LN, LN], bf16, kind="ExternalInput")
    ones = nc.dram_tensor("ones", [LN, 1], bf16, kind="ExternalInput")
    resf = nc.dram_tensor("resf", [2, 64], f32, kind="ExternalOutput")
    resc = nc.dram_tensor("resc", [1, NCOLS], f32, kind="ExternalOutput")

    with tile.TileContext(nc) as tc:
        with (
            tc.tile_pool(name="big", bufs=1) as big,
            tc.tile_pool(name="stage", bufs=3) as stage_p,
            tc.tile_pool(name="consts", bufs=1) as consts,
            tc.tile_pool(name="un", bufs=3) as unpool,
            tc.tile_pool(name="fin", bufs=1) as fin,
            tc.tile_pool(name="ps", bufs=2, space="PSUM") as pspool,
        ):
            el_sb = big.tile([LN, TEXK, NCOLS], bf16)
            s_sb = consts.tile([LN, LN], bf16)
            ones_sb = consts.tile([LN, 1], bf16)
            biasc = consts.tile([128, 1], f32)
            nc.vector.memset(biasc[:], -CSHIFT)

            # warm the ACT Exp/Ln tables while the first DMAs are in flight
            warm = consts.tile([1, 2], f32)
            nc.scalar.activation(warm[:, 0:1], biasc[0:1, :], AF.Exp)
            nc.scalar.activation(warm[:, 1:2], warm[:, 0:1], AF.Ln)

            nc.sync.dma_start(out=s_sb[:], in_=sm[:])
            nc.sync.dma_start(out=ones_sb[:], in_=ones[:])

            # emissions: stage bf16 logits, bulk-exp into el_sb, ascending k.
            # three aligned partition bands: fwd labels [0:64] x cols 0:64,
            # bwd labels [64:96] x all cols, sink rows [96:99] x all cols.
            for t0, n in EL_WINDOWS:
                st = stage_p.tile([LN, STAGE_MAX, NCOLS], bf16, tag="stage")
                nc.sync.dma_start(
                    out=st[0:64, 0:n, 0:64], in_=lg[0:64, t0 : t0 + n, 0:64]
                )
                nc.sync.dma_start(out=st[64:96, 0:n, :], in_=lg[64:96, t0 : t0 + n, :])
                nc.sync.dma_start(out=st[96:LN, 0:n, :], in_=lg[96:LN, t0 : t0 + n, :])
                nc.scalar.activation(
                    el_sb[0:64, t0 : t0 + n, 0:64],
                    st[0:64, 0:n, 0:64],
                    AF.Exp,
                    bias=biasc[0:64, :],
                )
                nc.scalar.activation(
                    el_sb[64:96, t0 : t0 + n, :],
                    st[64:96, 0:n, :],
                    AF.Exp,
                    bias=biasc[64:96, :],
                )
                nc.scalar.activation(
                    el_sb[96:LN, t0 : t0 + n, :],
                    st[96:LN, 0:n, :],
                    AF.Exp,
                    bias=biasc[96:LN, :],
                )

            # slice 0 doubles as the init state; clean its never-DMA'd quadrant
            nc.vector.memset(el_sb[0:64, 0, 64:NCOLS], 0.0)

            # stationary loaded once; every chain matmul skips the reload
            nc.tensor.ldweights(s_sb[:])

            # pre-create the 3 rotating state buffers zeroed so the unused
            # fwd quadrant (rows 0:64, cols 64:128) stays clean forever
            for _ in range(3):
                z = unpool.tile([LN, NCOLS], bf16, tag="un")
                nc.vector.memset(z[:], 0.0)

            state = el_sb[:, 0, :]
            un_last = None
            ps_last = None
            for k in range(1, KT + 1):
                pss = []
                for l in range(NLANE):
                    c0, c1 = CWID * l, CWID * l + CWID
                    ps = pspool.tile([LN, CWID], f32, tag=f"ps{l}")
                    mm = nc.tensor.matmul(
                        ps[:], s_sb[:], state[:, c0:c1], start=True, stop=True
                    )
                    mm.ldweights = False
                    pss.append(ps)
                un = unpool.tile([LN, NCOLS], bf16, tag="un")
                for l in range(NLANE):
                    c0, c1 = CWID * l, CWID * l + CWID
                    if l < 2:
                        nc.vector.tensor_mul(
                            un[:, c0:c1], pss[l][:], el_sb[:, k, c0:c1]
                        )
                    else:
                        nc.vector.tensor_mul(
                            un[64:LN, c0:c1],
                            pss[l][64:LN, :],
                            el_sb[64:LN, k, c0:c1],
                        )
                state = un[:]
                if k == KT:
                    un_last, ps_last = un, pss

            # ---- combine: Z = Σ_j u_256[j] β_256[j] for len > 256 ----
            # align fwd label finals (blocks A/B, lanes 0-1) onto bwd rows
            calign = fin.tile([LN, NCOLS], bf16, tag="calign")
            nc.scalar.activation(calign[64:96, 0:32], un_last[0:32, 0:32], AF.Copy)
            nc.scalar.activation(calign[64:96, 32:64], un_last[32:64, 0:32], AF.Copy)
            nc.scalar.activation(calign[64:96, 64:96], un_last[0:32, 32:64], AF.Copy)
            nc.scalar.activation(calign[64:96, 96:128], un_last[32:64, 32:64], AF.Copy)
            wt = fin.tile([LN, NCOLS], bf16, tag="wt")
            for l in range(NLANE):
                c0, c1 = CWID * l, CWID * l + CWID
                nc.vector.tensor_mul(
                    wt[64:96, c0:c1], ps_last[l][64:96, :], calign[64:96, c0:c1]
                )
            pscs = []
            for l in range(NLANE):
                c0, c1 = CWID * l, CWID * l + CWID
                psc = pspool.tile([LN, CWID], f32, tag=f"ps{l}")
                nc.tensor.matmul(
                    psc[0:1, :], ones_sb[64:96, :], wt[64:96, c0:c1],
                    start=True, stop=True,
                )
                pscs.append(psc)

            # resf = ln(fwd sinks): row 0 = A sinks, row 1 = B sinks
            rf = fin.tile([2, 64], f32, tag="rf")
            nc.scalar.activation(rf[0:2, 0:32], un_last[96:98, 0:32], AF.Ln)
            nc.scalar.activation(rf[0:2, 32:64], un_last[96:98, 32:64], AF.Ln)
            nc.sync.dma_start(out=resf[:], in_=rf[:])
            rc = fin.tile([1, NCOLS], f32, tag="rc")
            for l in range(NLANE):
                c0, c1 = CWID * l, CWID * l + CWID
                nc.scalar.activation(rc[:, c0:c1], pscs[l][0:1, :], AF.Ln)
            nc.sync.dma_start(out=resc[:], in_=rc[:])

    nc.compile()
    return nc


def _host_prep(logits, trans, labels, seq_lens):
    logits = np.ascontiguousarray(np.asarray(logits), dtype=np.float32)
    trans = np.asarray(trans, dtype=np.float32)
    labels = np.asarray(labels)
    lens = np.clip(np.asarray(seq_lens), 1, T).astype(np.int64)
    bf = ml_dtypes.bfloat16

    # ---- gold path score (host: index gathers over small inputs) ----
    tmask = np.arange(T)[None, :] < lens[:, None]
    unary = np.take_along_axis(logits, labels[..., None].astype(np.int64), axis=2)[..., 0]
    gp = (unary * tmask).sum(1) + (trans[labels[:, :-1], labels[:, 1:]] * tmask[:, 1:]).sum(1)

    # ---- device emission pack: [99 rows, k=0..256, 128 seq-cols] ----
    # rows 0:32 fwd labels block A, 32:64 block B, 64:96 bwd labels,
    # 96/97 fwd sinks A/B, 98 bwd sink.
    lgx = logits.copy()
    lgx[~tmask] = -1e9
    # sink emission indicator in log space, pre-compensated for the exp
    # bias so the on-device exp(x - CSHIFT) yields exactly 1.0 (or 0.0)
    el32log = np.where(
        np.arange(513)[None, :] >= lens[:, None], CSHIFT, -1e9
    ).astype(np.float32)

    cores = []
    for core in range(NCORES):
        b0 = core * BPC
        ll = lgx[b0 : b0 + BPC]            # [128, 512, 32]
        sl = el32log[b0 : b0 + BPC]        # [128, 513]
        arr = np.full((LN, TEXK, NCOLS), -1e9, np.float32)
        At = ll[:, 0:TEXK, :].transpose(2, 1, 0)   # [32, 257, 128]
        for lane in (0, 1):
            for blk in (0, 1):
                s0 = 64 * lane + 32 * blk
                c = slice(32 * lane, 32 * lane + 32)
                arr[32 * blk : 32 * blk + 32, :, c] = At[:, :, s0 : s0 + 32]
                arr[96 + blk, :, c] = sl[s0 : s0 + 32, 0:TEXK].T
        arr[64:96, 1:256, :] = ll[:, 511:256:-1, :].transpose(2, 1, 0)
        arr[98, 1:256, :] = sl[:, 511:256:-1].T
        arr[98, 0, :] = CSHIFT         # bwd init: sink state = 1 after exp
        cores.append(arr.astype(bf))

    # ---- stationary block-diag S and the combine colsum vector ----
    E = np.exp(trans).astype(np.float32)
    S = np.zeros((LN, LN), np.float32)
    S[0:32, 0:32] = E          # fwd block A labels
    S[32:64, 32:64] = E        # fwd block B labels
    S[64:96, 64:96] = E.T      # bwd labels
    S[0:32, 96] = 1.0          # A labels -> A sink
    S[96, 96] = 1.0
    S[32:64, 97] = 1.0         # B labels -> B sink
    S[97, 97] = 1.0
    S[98, 64:96] = 1.0         # bwd sink births beta=1 into labels
    S[98, 98] = 1.0
    ones = np.zeros((LN, 1), np.float32)
    ones[64:96, 0] = 1.0
    return gp, lens, cores, S.astype(bf), ones.astype(bf)


def _log(msg):
    import time as _t

    print(f"[kernel {_t.strftime('%H:%M:%S')}] {msg}", flush=True)


def kernel(logits, trans, labels, seq_lens):
    global last_result
    from concourse.bass_utils import run_bass_kernel_spmd

    _log("host prep start")
    gp, lens, cores, S, ones = _host_prep(logits, trans, labels, seq_lens)
    _log("host prep done")

    if "nc" not in _prog_cache:
        _prog_cache["nc"] = _build_program()
        _log("program built")
    nc = _prog_cache["nc"]

    in_maps = [
        {"lg": cores[i], "sm": S, "ones": ones} for i in range(NCORES)
    ]
    r = run_bass_kernel_spmd(nc, in_maps, core_ids=list(range(NCORES)))
    last_result = r
    _log("device run done")

    # ---- unshard + select sink vs combine per sequence length ----
    devf = np.zeros(B, np.float32)
    devc = np.zeros(B, np.float32)
    for core in range(NCORES):
        rf = r.results[core]["resf"]          # [2, 64]: [blk, 32*lane + s%32]
        rc = r.results[core]["resc"][0]       # [128]: col = seq
        b0 = core * BPC
        devc[b0 : b0 + BPC] = rc
        for lane in (0, 1):
            for blk in (0, 1):
                s0 = 64 * lane + 32 * blk
                devf[b0 + s0 : b0 + s0 + 32] = rf[blk, 32 * lane : 32 * lane + 32]

    dev = np.where(lens <= MEET, devf, devc)
    logZ = dev + CSHIFT * lens.astype(np.float32)
    return (gp - logZ).astype(np.float32)


# revision 25
# speedup vs baseline: 1.0023x; 1.0023x over previous
"""CRF log-likelihood kernel for Trainium2 (Bass/Tile), 8-core data parallel.

out[b] = gold_path_score(b) - logZ(b)

logZ via exp-domain DP, fwd and bwd chains meeting at t = 256:
  fwd:  u_k    = el_k ⊙ (Wf^T u_{k-1}),   k = 1..256   (u_0 = el_0)
  bwd:  γ̃_σ   = el_{T-σ} ⊙ (Wb^T γ̃_{σ-1}), σ = 1..255  (γ̃_0 = sink one)
  Z    = Σ_j u_256[j] · (Wb^T γ̃_255)[j]   (len > 256; sink capture else)
Both chains have the same mm-then-mul dataflow, so all work is 4 uniform
"lanes" of 32 columns against ONE block-diagonal stationary
S = diag(WfG, WfG, WbG) loaded into the PE array exactly once
(ldweights=False on every chain matmul). Each tick: 4 matmuls into one
shared PSUM tile + 4 DVE multiplies into one shared state tile, so the
framework emits ~1 WAR guard per tick instead of 4.

Partition layout (99 rows): [0:33]=fwd block A (32 labels + sink),
[33:66]=fwd block B, [66:99]=bwd block C. Columns: lane l = cols
32l..32l+32. Fwd state of seq s lives at (lane s//64, block (s%64)//32,
col s%32); bwd state of seq s at (lane s//32, block C, col s%32).
Emissions are host-packed per (row, k, col): rows 0-65 carry fwd times k,
rows 66-98 carry bwd times T-k — one ascending-k DMA/exp stream feeds
both chains. No renorm: CSHIFT centers the per-tick log-drift at ~0 and
the random walk stays inside bf16/f32 exponent range. Host adds
CSHIFT*len and does the gold-path gathers / final subtract.
"""

import numpy as np
import ml_dtypes

B, T, L = 1024, 512, 32
NCORES = 8
BPC = B // NCORES        # 128 sequences per core
LN = 99                  # partitions: 3 blocks x (32 labels + 1 sink)
NCOLS = 128              # columns = sequences per core
NLANE = 4
CWID = 32                # columns per lane
KT = 256                 # chain ticks
TEXK = 257               # el slices k = 0..256
MEET = 256               # host: len <= MEET -> sink path, else combine
CSHIFT = 4.5
EL_WINDOWS = [(0, 4), (4, 8), (12, 16), (28, 32), (60, 64), (124, 64), (188, 64), (252, 5)]
STAGE_MAX = max(n for _, n in EL_WINDOWS)

_prog_cache = {}
last_result = None       # BassKernelResults of the most recent run (for test.py)


def _build_program():
    import concourse.bacc as bacc
    import concourse.tile as tile
    from concourse import mybir

    f32 = mybir.dt.float32
    bf16 = mybir.dt.bfloat16
    AF = mybir.ActivationFunctionType

    nc = bacc.Bacc("TRN2", target_bir_lowering=False, debug=False, num_devices=NCORES)
    lg = nc.dram_tensor("lg", [LN, TEXK, NCOLS], bf16, kind="ExternalInput")
    sm = nc.dram_tensor("sm", [LN, LN], bf16, kind="ExternalInput")
    ones = nc.dram_tensor("ones", [LN, 1], bf16, kind="ExternalInput")
    resf = nc.dram_tensor("resf", [2, 64], f32, kind="ExternalOutput")
    resc = nc.dram_tensor("resc", [1, NCOLS], f32, kind="ExternalOutput")

    with tile.TileContext(nc) as tc:
        with (
            tc.tile_pool(name="big", bufs=1) as big,
            tc.tile_pool(name="stage", bufs=3) as stage_p,
            tc.tile_pool(name="consts", bufs=1) as consts,
            tc.tile_pool(name="un", bufs=3) as unpool,
            tc.tile_pool(name="fin", bufs=1) as fin,
            tc.tile_pool(name="ps", bufs=2, space="PSUM") as pspool,
        ):
            el_sb = big.tile([LN, TEXK, NCOLS], bf16)
            s_sb = consts.tile([LN, LN], bf16)
            ones_sb = consts.tile([LN, 1], bf16)
            biasc = consts.tile([128, 1], f32)
            nc.vector.memset(biasc[:], -CSHIFT)

            # warm the ACT Exp/Ln tables while the first DMAs are in flight
            warm = consts.tile([1, 2], f32)
            nc.scalar.activation(warm[:, 0:1], biasc[0:1, :], AF.Exp)
            nc.scalar.activation(warm[:, 1:2], warm[:, 0:1], AF.Ln)

            nc.sync.dma_start(out=s_sb[:], in_=sm[:])
            nc.sync.dma_start(out=ones_sb[:], in_=ones[:])

            # emissions: stage bf16 logits, bulk-exp into el_sb, ascending k.
            # three aligned partition bands: fwd labels [0:64] x cols 0:64,
            # bwd labels [64:96] x all cols, sink rows [96:99] x all cols.
            for t0, n in EL_WINDOWS:
                st = stage_p.tile([LN, STAGE_MAX, NCOLS], bf16, tag="stage")
                nc.sync.dma_start(
                    out=st[0:64, 0:n, 0:64], in_=lg[0:64, t0 : t0 + n, 0:64]
                )
                nc.sync.dma_start(out=st[64:96, 0:n, :], in_=lg[64:96, t0 : t0 + n, :])
                nc.sync.dma_start(out=st[96:LN, 0:n, :], in_=lg[96:LN, t0 : t0 + n, :])
                nc.scalar.activation(
                    el_sb[0:64, t0 : t0 + n, 0:64],
                    st[0:64, 0:n, 0:64],
                    AF.Exp,
                    bias=biasc[0:64, :],
                )
                nc.scalar.activation(
                    el_sb[64:96, t0 : t0 + n, :],
                    st[64:96, 0:n, :],
                    AF.Exp,
                    bias=biasc[64:96, :],
                )
                nc.scalar.activation(
                    el_sb[96:LN, t0 : t0 + n, :],
                    st[96:LN, 0:n, :],
                    AF.Exp,
                    bias=biasc[96:LN, :],
                )

            # slice 0 doubles as the init state; clean its never-DMA'd quadrant
            nc.vector.memset(el_sb[0:64, 0, 64:NCOLS], 0.0)

            # stationary loaded once; every chain matmul skips the reload
            nc.tensor.ldweights(s_sb[:])

            # pre-create the 3 rotating state buffers zeroed so the unused
            # fwd quadrant (rows 0:64, cols 64:128) stays clean forever
            for _ in range(3):
                z = unpool.tile([LN, NCOLS], bf16, tag="un")
                nc.vector.memset(z[:], 0.0)

            state = el_sb[:, 0, :]
            un_last = None
            ps_last = None
            for k in range(1, KT + 1):
                pss = []
                for l in range(NLANE):
                    c0, c1 = CWID * l, CWID * l + CWID
                    ps = pspool.tile([LN, CWID], f32, tag=f"ps{l}")
                    mm = nc.tensor.matmul(
                        ps[:], s_sb[:], state[:, c0:c1], start=True, stop=True
                    )
                    mm.ins.ldweights = False
                    pss.append(ps)
                un = unpool.tile([LN, NCOLS], bf16, tag="un")
                for l in range(NLANE):
                    c0, c1 = CWID * l, CWID * l + CWID
                    if l < 2:
                        nc.vector.tensor_mul(
                            un[:, c0:c1], pss[l][:], el_sb[:, k, c0:c1]
                        )
                    else:
                        nc.vector.tensor_mul(
                            un[64:LN, c0:c1],
                            pss[l][64:LN, :],
                            el_sb[64:LN, k, c0:c1],
                        )
                state = un[:]
                if k == KT:
                    un_last, ps_last = un, pss

            # ---- combine: Z = Σ_j u_256[j] β_256[j] for len > 256 ----
            # align fwd label finals (blocks A/B, lanes 0-1) onto bwd rows
            calign = fin.tile([LN, NCOLS], bf16, tag="calign")
            nc.scalar.activation(calign[64:96, 0:32], un_last[0:32, 0:32], AF.Copy)
            nc.scalar.activation(calign[64:96, 32:64], un_last[32:64, 0:32], AF.Copy)
            nc.scalar.activation(calign[64:96, 64:96], un_last[0:32, 32:64], AF.Copy)
            nc.scalar.activation(calign[64:96, 96:128], un_last[32:64, 32:64], AF.Copy)
            wt = fin.tile([LN, NCOLS], bf16, tag="wt")
            for l in range(NLANE):
                c0, c1 = CWID * l, CWID * l + CWID
                nc.vector.tensor_mul(
                    wt[64:96, c0:c1], ps_last[l][64:96, :], calign[64:96, c0:c1]
                )
            pscs = []
            for l in range(NLANE):
                c0, c1 = CWID * l, CWID * l + CWID
                psc = pspool.tile([LN, CWID], f32, tag=f"ps{l}")
                nc.tensor.matmul(
                    psc[0:1, :], ones_sb[64:96, :], wt[64:96, c0:c1],
                    start=True, stop=True,
                )
                pscs.append(psc)

            # resf = ln(fwd sinks): row 0 = A sinks, row 1 = B sinks
            rf = fin.tile([2, 64], f32, tag="rf")
            nc.scalar.activation(rf[0:2, 0:32], un_last[96:98, 0:32], AF.Ln)
            nc.scalar.activation(rf[0:2, 32:64], un_last[96:98, 32:64], AF.Ln)
            nc.sync.dma_start(out=resf[:], in_=rf[:])
            rc = fin.tile([1, NCOLS], f32, tag="rc")
            for l in range(NLANE):
                c0, c1 = CWID * l, CWID * l + CWID
                nc.scalar.activation(rc[:, c0:c1], pscs[l][0:1, :], AF.Ln)
            nc.sync.dma_start(out=resc[:], in_=rc[:])

    nc.compile()
    return nc


def _host_prep(logits, trans, labels, seq_lens):
    logits = np.ascontiguousarray(np.asarray(logits), dtype=np.float32)
    trans = np.asarray(trans, dtype=np.float32)
    labels = np.asarray(labels)
    lens = np.clip(np.asarray(seq_lens), 1, T).astype(np.int64)
    bf = ml_dtypes.bfloat16

    # ---- gold path score (host: index gathers over small inputs) ----
    tmask = np.arange(T)[None, :] < lens[:, None]
    unary = np.take_along_axis(logits, labels[..., None].astype(np.int64), axis=2)[..., 0]
    gp = (unary * tmask).sum(1) + (trans[labels[:, :-1], labels[:, 1:]] * tmask[:, 1:]).sum(1)

    # ---- device emission pack: [99 rows, k=0..256, 128 seq-cols] ----
    # rows 0:32 fwd labels block A, 32:64 block B, 64:96 bwd labels,
    # 96/97 fwd sinks A/B, 98 bwd sink.
    lgx = logits.copy()
    lgx[~tmask] = -1e9
    # sink emission indicator in log space, pre-compensated for the exp
    # bias so the on-device exp(x - CSHIFT) yields exactly 1.0 (or 0.0)
    el32log = np.where(
        np.arange(513)[None, :] >= lens[:, None], CSHIFT, -1e9
    ).astype(np.float32)

    cores = []
    for core in range(NCORES):
        b0 = core * BPC
        ll = lgx[b0 : b0 + BPC]            # [128, 512, 32]
        sl = el32log[b0 : b0 + BPC]        # [128, 513]
        arr = np.full((LN, TEXK, NCOLS), -1e9, np.float32)
        At = ll[:, 0:TEXK, :].transpose(2, 1, 0)   # [32, 257, 128]
        for lane in (0, 1):
            for blk in (0, 1):
                s0 = 64 * lane + 32 * blk
                c = slice(32 * lane, 32 * lane + 32)
                arr[32 * blk : 32 * blk + 32, :, c] = At[:, :, s0 : s0 + 32]
                arr[96 + blk, :, c] = sl[s0 : s0 + 32, 0:TEXK].T
        arr[64:96, 1:256, :] = ll[:, 511:256:-1, :].transpose(2, 1, 0)
        arr[98, 1:256, :] = sl[:, 511:256:-1].T
        arr[98, 0, :] = CSHIFT         # bwd init: sink state = 1 after exp
        cores.append(arr.astype(bf))

    # ---- stationary block-diag S and the combine colsum vector ----
    E = np.exp(trans).astype(np.float32)
    S = np.zeros((LN, LN), np.float32)
    S[0:32, 0:32] = E          # fwd block A labels
    S[32:64, 32:64] = E        # fwd block B labels
    S[64:96, 64:96] = E.T      # bwd labels
    S[0:32, 96] = 1.0          # A labels -> A sink
    S[96, 96] = 1.0
    S[32:64, 97] = 1.0         # B labels -> B sink
    S[97, 97] = 1.0
    S[98, 64:96] = 1.0         # bwd sink births beta=1 into labels
    S[98, 98] = 1.0
    ones = np.zeros((LN, 1), np.float32)
    ones[64:96, 0] = 1.0
    return gp, lens, cores, S.astype(bf), ones.astype(bf)


def _log(msg):
    import time as _t

    print(f"[kernel {_t.strftime('%H:%M:%S')}] {msg}", flush=True)


def kernel(logits, trans, labels, seq_lens):
    global last_result
    from concourse.bass_utils import run_bass_kernel_spmd

    _log("host prep start")
    gp, lens, cores, S, ones = _host_prep(logits, trans, labels, seq_lens)
    _log("host prep done")

    if "nc" not in _prog_cache:
        _prog_cache["nc"] = _build_program()
        _log("program built")
    nc = _prog_cache["nc"]

    in_maps = [
        {"lg": cores[i], "sm": S, "ones": ones} for i in range(NCORES)
    ]
    r = run_bass_kernel_spmd(nc, in_maps, core_ids=list(range(NCORES)))
    last_result = r
    _log("device run done")

    # ---- unshard + select sink vs combine per sequence length ----
    devf = np.zeros(B, np.float32)
    devc = np.zeros(B, np.float32)
    for core in range(NCORES):
        rf = r.results[core]["resf"]          # [2, 64]: [blk, 32*lane + s%32]
        rc = r.results[core]["resc"][0]       # [128]: col = seq
        b0 = core * BPC
        devc[b0 : b0 + BPC] = rc
        for lane in (0, 1):
            for blk in (0, 1):
                s0 = 64 * lane + 32 * blk
                devf[b0 + s0 : b0 + s0 + 32] = rf[blk, 32 * lane : 32 * lane + 32]

    dev = np.where(lens <= MEET, devf, devc)
    logZ = dev + CSHIFT * lens.astype(np.float32)
    return (gp - logZ).astype(np.float32)


# revision 27
# speedup vs baseline: 1.3464x; 1.3434x over previous
"""CRF log-likelihood kernel for Trainium2 (Bass/Tile), 8-core data parallel.

out[b] = gold_path_score(b) - logZ(b)

logZ via exp-domain DP with forward and backward chains meeting at t = F:
  fwd:  u_t   = el_t  ⊙ (Wf^T u_{t-1}),      t = 1..F      (u_0 = el_0)
  bwd:  γ_σ   = Wb^T (el_{T+1-σ} ⊙ γ_{σ-1}), σ = 1..T-F    (γ_0 = sink)
Sequences with len <= F finish inside the fwd chain via an absorbing "sink"
label that captures sum_i u_{len-1}[i] exactly at t == len; longer sequences
use the midpoint identity Z = Σ_j α_F[j]·β_F[j], with the bwd chain's sink
"birthing" β = 1 at each sequence's own end time. The two chains are
independent, so PE matmuls of one overlap DVE multiplies of the other.

Layout per core (128 sequences):
  partitions 0..95 = active labels (3 groups x 32), 96..98 = sink row per
  group; psum rows 99..101 = per-group column sums (ones-columns of the
  stationary operand). columns: b_local = 43*g + c.
Scaling: all emissions carry e^{-CSHIFT}; columns are renormalized by their
column sum mid-chain (factor tracked exactly via ACT-Ln of the applied
multiplier). Host adds CSHIFT*len back and picks sink vs combine per length.
Host also does the gold-path gathers (labels/trans only) and final subtract.
"""

import numpy as np
import ml_dtypes

B, T, L = 1024, 512, 32
NCORES = 8
BPC = B // NCORES        # 128 sequences per core
G = 3                    # label groups per core
NCOL = 43                # columns per group (group 2 uses 42 + 1 pad)
NACT = 96                # active label partitions
NPART = 99               # + 3 sink rows
MOUT = 102               # + 3 colsum rows
CSHIFT = 4.5
TEX = T + 1              # el time slices 0..T
F = 256                  # fwd ticks; bwd ticks = T - F
SB = T - F
RENORM_EVERY = 128


def _el_windows():
    """Graded (t0, n) windows covering [0, TEX), smallest first, alternating
    tail (bwd consumes from t=T down) and head (fwd from t=0 up) so both
    chains can start after ~2 small DMAs instead of waiting out 1MB chunks."""
    sizes = [8, 16, 32, 64, 96]
    head, tail = [], []
    lo, hi = 0, TEX
    for s in sizes:
        tail.append((hi - s, s)); hi -= s
        head.append((lo, s)); lo += s
    # remainder split once more (tail gets the first share)
    rem = hi - lo
    a = rem // 2
    tail.append((hi - a, a)); hi -= a
    head.append((lo, hi - lo))
    order = []
    for tl, hd in zip(tail, head):
        order.append(tl); order.append(hd)
    return order


EL_WINDOWS = _el_windows()
STAGE_MAX = max(n for _, n in EL_WINDOWS)

_prog_cache = {}
last_result = None       # BassKernelResults of the most recent run (for test.py)


def _build_program():
    import concourse.bacc as bacc
    import concourse.tile as tile
    from concourse import mybir

    f32 = mybir.dt.float32
    bf16 = mybir.dt.bfloat16
    AF = mybir.ActivationFunctionType

    nc = bacc.Bacc("TRN2", target_bir_lowering=False, debug=False, num_devices=NCORES)
    lg = nc.dram_tensor("lg", [NACT, TEX, NCOL], f32, kind="ExternalInput")
    el32 = nc.dram_tensor("el32", [G, TEX, NCOL], bf16, kind="ExternalInput")
    wf = nc.dram_tensor("wf", [NPART, MOUT], bf16, kind="ExternalInput")
    wbk = nc.dram_tensor("wbk", [NPART, MOUT], bf16, kind="ExternalInput")
    wcs = nc.dram_tensor("wcs", [NPART, G], bf16, kind="ExternalInput")
    resf = nc.dram_tensor("resf", [G, NCOL], f32, kind="ExternalOutput")
    resc = nc.dram_tensor("resc", [G, NCOL], f32, kind="ExternalOutput")

    with tile.TileContext(nc) as tc:
        with (
            tc.tile_pool(name="big", bufs=1) as big,
            tc.tile_pool(name="stage", bufs=3) as stage_p,
            tc.tile_pool(name="consts", bufs=1) as consts,
            tc.tile_pool(name="u", bufs=3) as upool,
            tc.tile_pool(name="v", bufs=3) as vpool,
            tc.tile_pool(name="fin", bufs=1) as fin,
            tc.tile_pool(name="psf", bufs=3, space="PSUM") as psfpool,
            tc.tile_pool(name="psb", bufs=3, space="PSUM") as psbpool,
        ):
            el_sb = big.tile([NPART, TEX, NCOL], bf16)
            wf_sb = consts.tile([NPART, MOUT], bf16)
            wb_sb = consts.tile([NPART, MOUT], bf16)
            wcs_sb = consts.tile([NPART, G], bf16)
            biasc = consts.tile([128, 1], f32)
            g0 = consts.tile([NPART, NCOL], bf16)
            nc.vector.memset(biasc[:], -CSHIFT)
            nc.vector.memset(g0[:], 0.0)
            nc.vector.memset(g0[NACT:NPART, :], 1.0)

            # warm the ACT Exp/Ln tables while the first DMAs are in flight
            warm = consts.tile([1, 2], f32)
            nc.scalar.activation(warm[:, 0:1], biasc[0:1, :], AF.Exp)
            nc.scalar.activation(warm[:, 1:2], warm[:, 0:1], AF.Ln)

            nc.sync.dma_start(out=wf_sb[:], in_=wf[:])
            nc.sync.dma_start(out=wb_sb[:], in_=wbk[:])
            nc.sync.dma_start(out=wcs_sb[:], in_=wcs[:])
            # active rows: stage raw logits, bulk-exp into el_sb.
            # graded windows, alternating ends: bwd consumes from t=T down.
            # sink rows (partitions 96..98) stream in the same windows so no
            # monolithic descriptor hogs a DMA engine ahead of the first chunks.
            for t0, n in EL_WINDOWS:
                st = stage_p.tile([NACT, STAGE_MAX, NCOL], f32, tag="stage")
                nc.sync.dma_start(out=st[:, 0:n, :], in_=lg[:, t0 : t0 + n, :])
                nc.sync.dma_start(
                    out=el_sb[NACT:NPART, t0 : t0 + n, :], in_=el32[:, t0 : t0 + n, :]
                )
                nc.scalar.activation(
                    el_sb[0:NACT, t0 : t0 + n, :],
                    st[:, 0:n, :],
                    AF.Exp,
                    bias=biasc[0:NACT, :],
                )

            # Four independent latency lanes: fwd/bwd x column halves.
            # No renorm: CSHIFT ~ log(L * E[e^trans] * E[e^logit]) makes the
            # expected per-tick log-drift ~0; the +-4 sigma random walk over
            # 256 ticks stays well inside bf16/f32 exponent range.
            CW = [(0, NCOL)]
            uprev = [el_sb[:, 0, c0:c1] for c0, c1 in CW]
            gprev = [g0[:, c0:c1] for c0, c1 in CW]
            gprev_sbuf = [True, True]
            ulast = [None, None]
            pb_last = [None, None]
            for k in range(1, max(F, SB) + 1):
                # ---- fwd tick t = k (both column halves) ----
                if k <= F:
                    psfs = []
                    for h, (c0, c1) in enumerate(CW):
                        psf = psfpool.tile([MOUT, c1 - c0], f32, tag=f"psf{h}")
                        nc.tensor.matmul(psf[:], wf_sb[:], uprev[h], start=True, stop=True)
                        psfs.append(psf)
                    for h, (c0, c1) in enumerate(CW):
                        un = upool.tile([NPART, c1 - c0], bf16, tag=f"u{h}")
                        nc.vector.tensor_mul(un[:], psfs[h][0:NPART, :], el_sb[:, k, c0:c1])
                        uprev[h] = un[:]
                    if k == F:
                        ulast = list(uprev)
                # ---- bwd tick σ = k, el time T+1-k (both column halves) ----
                if k <= SB:
                    vns = []
                    for h, (c0, c1) in enumerate(CW):
                        vn = vpool.tile([NPART, c1 - c0], bf16, tag=f"v{h}")
                        src = gprev[h] if gprev_sbuf[h] else gprev[h][0:NPART, :]
                        nc.vector.tensor_mul(vn[:], src, el_sb[:, T + 1 - k, c0:c1])
                        vns.append(vn)
                    for h, (c0, c1) in enumerate(CW):
                        psb = psbpool.tile([MOUT, c1 - c0], f32, tag=f"psb{h}")
                        nc.tensor.matmul(psb[:], wb_sb[:], vns[h][:], start=True, stop=True)
                        gprev[h] = psb
                        gprev_sbuf[h] = False
                    if k == SB:
                        pb_last = [(gprev[h], gprev_sbuf[h]) for h in range(len(CW))]

            # ---- combine: w = u_F ⊙ γ_S; Zc = per-group colsum of w ----
            accf = fin.tile([G, NCOL], f32, tag="lnu")
            accc = fin.tile([G, NCOL], f32, tag="lnc")
            for h, (c0, c1) in enumerate(CW):
                gl, gl_sbuf = pb_last[h]
                wt = vpool.tile([NPART, c1 - c0], bf16, tag=f"wt{h}")
                nc.vector.tensor_mul(wt[:], gl if gl_sbuf else gl[0:NPART, :], ulast[h])
                psc = psfpool.tile([MOUT, c1 - c0], f32, tag=f"psf{h}")
                nc.tensor.matmul(psc[0:G, :], wcs_sb[:], wt[:], start=True, stop=True)
                nc.scalar.activation(accf[:, c0:c1], ulast[h][NACT:NPART, :], AF.Ln)
                nc.scalar.activation(accc[:, c0:c1], psc[0:G, :], AF.Ln)
            nc.sync.dma_start(out=resf[:], in_=accf[:])
            nc.sync.dma_start(out=resc[:], in_=accc[:])

    nc.compile()
    return nc


def _host_prep(logits, trans, labels, seq_lens):
    logits = np.ascontiguousarray(np.asarray(logits), dtype=np.float32)
    trans = np.asarray(trans, dtype=np.float32)
    labels = np.asarray(labels)
    lens = np.clip(np.asarray(seq_lens), 1, T).astype(np.int64)

    # ---- gold path score (host: index gathers over small inputs) ----
    tmask = np.arange(T)[None, :] < lens[:, None]
    unary = np.take_along_axis(logits, labels[..., None].astype(np.int64), axis=2)[..., 0]
    gp = (unary * tmask).sum(1) + (trans[labels[:, :-1], labels[:, 1:]] * tmask[:, 1:]).sum(1)

    # ---- device inputs: mask every t >= len; pad slice t=T = -inf ----
    lgx = logits.copy()
    lgx[~tmask] = -1e9
    lgx = np.concatenate([lgx, np.full((B, 1, L), -1e9, np.float32)], axis=1)

    el32 = (np.arange(TEX)[None, :] >= lens[:, None]).astype(np.float32)  # [B, 513]

    lg_cores, el32_cores = [], []
    for core in range(NCORES):
        b0 = core * BPC
        lgp = np.full((G, 32, TEX, NCOL), -1e9, np.float32)
        e32 = np.zeros((G, TEX, NCOL), np.float32)
        for g in range(G):
            ncols = NCOL if g < 2 else BPC - 2 * NCOL
            bs = b0 + g * NCOL
            lgp[g, :, :, :ncols] = lgx[bs : bs + ncols].transpose(2, 1, 0)
            e32[g, :, :ncols] = el32[bs : bs + ncols].T
            if ncols < NCOL:  # pad column: dummy len==T sequence, active el = 0
                e32[g, T, ncols:] = 1.0
        lg_cores.append(np.ascontiguousarray(lgp).reshape(NACT, TEX, NCOL))
        el32_cores.append(e32.astype(ml_dtypes.bfloat16))

    # ---- stationary operators ----
    E = np.exp(trans).astype(np.float32)
    Wf = np.zeros((NPART, MOUT), np.float32)
    Wb = np.zeros((NPART, MOUT), np.float32)
    Wcs = np.zeros((NPART, G), np.float32)
    for g in range(G):
        a, sk, cs = 32 * g, NACT + g, NPART + g
        Wf[a : a + 32, a : a + 32] = E
        Wf[a : a + 32, sk] = 1.0
        Wf[sk, sk] = 1.0
        Wf[a : a + 32, cs] = 1.0
        Wf[sk, cs] = 1.0
        Wb[a : a + 32, a : a + 32] = E.T
        Wb[sk, a : a + 32] = 1.0   # sink births β = 1 over all labels
        Wb[sk, sk] = 1.0
        Wb[a : a + 32, cs] = 1.0
        Wb[sk, cs] = 1.0
        Wcs[a : a + 32, g] = 1.0
        Wcs[sk, g] = 1.0
    bf = ml_dtypes.bfloat16
    return gp, lens, lg_cores, el32_cores, Wf.astype(bf), Wb.astype(bf), Wcs.astype(bf)


def _log(msg):
    import time as _t

    print(f"[kernel {_t.strftime('%H:%M:%S')}] {msg}", flush=True)


def kernel(logits, trans, labels, seq_lens):
    global last_result
    from concourse.bass_utils import run_bass_kernel_spmd

    _log("host prep start")
    gp, lens, lg_cores, el32_cores, Wf, Wb, Wcs = _host_prep(
        logits, trans, labels, seq_lens
    )
    _log("host prep done")

    if "nc" not in _prog_cache:
        _prog_cache["nc"] = _build_program()
        _log("program built")
    nc = _prog_cache["nc"]

    in_maps = [
        {
            "lg": lg_cores[i],
            "el32": el32_cores[i],
            "wf": Wf,
            "wbk": Wb,
            "wcs": Wcs,
        }
        for i in range(NCORES)
    ]
    r = run_bass_kernel_spmd(nc, in_maps, core_ids=list(range(NCORES)))
    last_result = r
    _log("device run done")

    # ---- unshard + select sink vs combine per sequence length ----
    devf = np.zeros(B, np.float32)
    devc = np.zeros(B, np.float32)
    for core in range(NCORES):
        rf = r.results[core]["resf"]
        rc = r.results[core]["resc"]
        b0 = core * BPC
        for g in range(G):
            ncols = NCOL if g < 2 else BPC - 2 * NCOL
            devf[b0 + g * NCOL : b0 + g * NCOL + ncols] = rf[g, :ncols]
            devc[b0 + g * NCOL : b0 + g * NCOL + ncols] = rc[g, :ncols]

    dev = np.where(lens <= F, devf, devc)
    logZ = dev + CSHIFT * lens.astype(np.float32)
    return (gp - logZ).astype(np.float32)

